# revision 43
# baseline (speedup 1.0000x reference)
"""Trainium2 Bass kernel for a dense transformer block (nn_Block_58377195487260).

Reference (per batch element, fp32):
    h   = LN1(x)*g1 + b1ln
    q,k,v = h@wq, h@wk, h@wv
    s   = q@k^T / sqrt(dk);  a = softmax(s);  y = (a@v)@wo
    x2  = h + y
    mlp = gelu(LN2(x2) @ w1 + b1) @ w2 + b2
    out = x2 + mlp

Sharding: data-parallel over batch. B=8 == 8 NeuronCores; core i computes
batch element i end-to-end (no collectives).

Numerics/performance strategy (PE-bound kernel, ~90% tensor occupancy):
  - QKV and the MLP run in bf16 (fp8 there fails the 2e-2 tolerance).
  - The attention core runs fp8-e4m3 with DoubleRow perf mode (2 k-chunks
    per pass): scores (qT8 x kT8), softmax sums (ones8 x ET), A@V (V8 x ET),
    and the wo projection (wo8 x yTs8).  exp is computed with a constant
    offset -2.5 so the unnormalized weights stay < 240 (TRN fp8e4 max);
    the offset cancels in the normalization.
  - LN2 statistics are computed during the attention phase (partition-sum
    matmuls on the freshly produced x2 tiles) so the MLP phase starts with
    rstd/mean broadcasts already in SBUF.
  - The MLP output is produced directly in natural [s, d] layout
    (stationary = GT chunks), with the x2 residual folded into the same
    PSUM accumulation via identity matmuls: no output transposes.
  - x2 spills to DRAM in bf16 (block 0 stays in SBUF) and is prefetched
    back during the MLP phase; w2 is loaded during phase 3 onto the right
    SBUF stack so it is resident when the attention pools release the left.
"""

import numpy as np
import ml_dtypes
from contextlib import ExitStack

P = 128
B, S, D, H = 8, 2048, 1024, 4096
DC = D // P          # 8  d-chunks
HC = H // P          # 32 h-chunks
SC = S // P          # 16 s-chunks
QB = 512             # attention sq-block == ln2/mlp s-block
NQB = S // QB        # 4
MB = 512
NMB = S // MB        # 4
EPS = 1e-5
SM_SCALE = 1.0 / 32.0   # 1/sqrt(1024)
EXP_OFF = -2.5          # exp offset; cancels in softmax normalization
# DoubleRow fp8 matmuls process two k-chunks per pass on the attention
# path.  (The PE clock varies run-to-run between ~2.0 and ~2.4 GHz
# independent of this flag; at equal clock DR saved ~115us.)
USE_DR = True

N_CORES = 8


def build(nc, bass, mybir, tile):
    f32 = mybir.dt.float32
    bf16 = mybir.dt.bfloat16
    f8 = mybir.dt.float8e4

    x_in = nc.declare_dram_parameter("x", [S, D], f32, isOutput=False)
    # pre-tiled [partition, in_chunk, out_col] so each load is contiguous
    wq_in = nc.declare_dram_parameter("wq", [P, DC, D], bf16, isOutput=False)
    wk_in = nc.declare_dram_parameter("wk", [P, DC, D], bf16, isOutput=False)
    wv_in = nc.declare_dram_parameter("wv", [P, DC, D], bf16, isOutput=False)
    wo_in = nc.declare_dram_parameter("wo", [P, DC, D], f8, isOutput=False)
    # w1 pre-scaled by ln2_g, tiled [hc][p][dc][n]; w1gs = -sum_d w1g2[d,h];
    # b1 arrives as b1 + w1.T @ ln2_b (LN2 folded into the GT matmul)
    w1_in = nc.declare_dram_parameter("w1", [HC, P, DC, P], bf16, isOutput=False)
    w1gs_in = nc.declare_dram_parameter("w1gs", [H], f32, isOutput=False)
    w2_in = nc.declare_dram_parameter("w2", [P, HC, D], bf16, isOutput=False)
    ln1g_in = nc.declare_dram_parameter("ln1_g", [D], f32, isOutput=False)
    ln1b_in = nc.declare_dram_parameter("ln1_b", [D], f32, isOutput=False)
    b1_in = nc.declare_dram_parameter("b1", [H], f32, isOutput=False)
    b2_in = nc.declare_dram_parameter("b2", [D], f32, isOutput=False)
    out_dram = nc.declare_dram_parameter("out", [S, D], f32, isOutput=True)

    from concourse.masks import make_identity

    with tile.TileContext(nc) as tc, ExitStack() as top:
        const = top.enter_context(tc.tile_pool(name="const", bufs=1))
        persist = top.enter_context(
            tc.tile_pool(name="persist", bufs=1, side="right"))
        dram = top.enter_context(tc.tile_pool(name="dram", bufs=1, space="DRAM"))

        ident_bf = const.tile([P, P], bf16)
        make_identity(nc, ident_bf)
        eps_p = const.tile([P, 1], f32)
        nc.vector.memset(eps_p, EPS)
        eps_1 = const.tile([1, 1], f32)
        nc.vector.memset(eps_1, EPS)
        ones_bf = const.tile([P, 1], bf16)
        nc.vector.memset(ones_bf, 1.0)
        expoff = const.tile([P, 1], f32)
        nc.vector.memset(expoff, EXP_OFF)
        # all-ones stationaries: partition sums land on every partition
        # (pre-broadcast), full-width M=128 so no FWL/M=1 penalties
        allones_bf = const.tile([P, P], bf16)
        nc.vector.memset(allones_bf, 1.0)
        allones8_t = const.tile([P, 2, P], f8)
        nc.vector.memset(allones8_t, 1.0)
        allones8 = allones8_t
        allones8_2 = allones8_t[:, 0, :]
        ones_row = const.tile([1, P], bf16)
        nc.vector.memset(ones_row, 1.0)

        # per-partition views of gains/biases: [P, nchunk], column c = chunk c
        ln1g = const.tile([P, DC], f32)
        ln1b = const.tile([P, DC], f32)
        b1c = const.tile([P, HC], f32)
        w1gs = const.tile([P, HC], f32)
        for dst, src in ((ln1g, ln1g_in), (ln1b, ln1b_in),
                         (b1c, b1_in), (w1gs, w1gs_in)):
            nc.sync.dma_start(out=dst, in_=src.rearrange("(c p) -> p c", p=P))

        # persistent across phase 3 -> 4/5
        x2b0 = persist.tile([P, DC, MB], bf16)     # x2 block 0 (stays on-chip)
        rsab = persist.tile([P, 2 * NMB, MB], bf16)  # rstd / mu*rstd bcast
        b2bc = persist.tile([P, D], bf16)          # b2 broadcast over partitions

        x2T_dram = dram.tile([P, DC, S], bf16)     # x2 spill, blocks 1..3

        import os
        for _rep in range(int(os.environ.get("BENCH_REPS", "1"))):
            _build_body(nc, tc, mybir, locals())

    nc.finalize()
    return nc


def _build_body(nc, tc, mybir, env):
    f32 = mybir.dt.float32
    bf16 = mybir.dt.bfloat16
    f8 = mybir.dt.float8e4
    AF = mybir.ActivationFunctionType
    ALU = mybir.AluOpType
    DR = mybir.MatmulPerfMode.DoubleRow
    (x_in, wq_in, wk_in, wv_in, wo_in, w1_in, w2_in, b2_in, out_dram,
     x2T_dram, ident_bf, eps_p, eps_1, ones_bf, ones_row,
     ln1g, ln1b, b1c, w1gs, x2b0, rsab, b2bc, expoff,
     allones_bf, allones8, allones8_2) = (
        env["x_in"], env["wq_in"], env["wk_in"], env["wv_in"], env["wo_in"],
        env["w1_in"], env["w2_in"], env["b2_in"], env["out_dram"],
        env["x2T_dram"], env["ident_bf"], env["eps_p"], env["eps_1"],
        env["ones_bf"], env["ones_row"], env["ln1g"],
        env["ln1b"], env["b1c"], env["w1gs"], env["x2b0"], env["rsab"],
        env["b2bc"], env["expoff"],
        env["allones_bf"], env["allones8"], env["allones8_2"])

    with ExitStack() as whole:
        w2_sb = None
        with ExitStack() as attn:
            act = attn.enter_context(tc.tile_pool(name="act", bufs=1))
            hT = act.tile([P, DC, S], bf16)          # [d, s] post-LN1
            qT = act.tile([P, DC, S], f8)            # [dk, s]
            kT = act.tile([P, DC, S], f8)
            V = act.tile([P, SC, D], f8)             # [s, dv]

            # ---- Phase 0-2 per 512-col s-block: LN1 + transpose + QKV ----
            with ExitStack() as ph:
                wp = ph.enter_context(tc.tile_pool(name="wp", bufs=1))
                xp = ph.enter_context(tc.tile_pool(name="xp", bufs=3))
                hbp = ph.enter_context(tc.tile_pool(name="hbp", bufs=3))
                st = ph.enter_context(tc.tile_pool(name="st", bufs=4))
                b2t = ph.enter_context(tc.tile_pool(name="b2t", bufs=1))
                tps = ph.enter_context(
                    tc.tile_pool(name="tps", bufs=4, space="PSUM"))
                mps = ph.enter_context(
                    tc.tile_pool(name="mps", bufs=4, space="PSUM"))

                # first x rows start streaming before the big weight loads
                x_ts = []
                for sc in range(4):
                    x_t = xp.tile([P, D], f32, tag="x")
                    nc.sync.dma_start(out=x_t,
                                      in_=x_in[sc * P:(sc + 1) * P, :])
                    x_ts.append(x_t)

                wq_sb = wp.tile([P, DC, D], bf16)
                wk_sb = wp.tile([P, DC, D], bf16)
                wv_sb = wp.tile([P, DC, D], bf16)
                for dst, src in ((wq_sb, wq_in), (wk_sb, wk_in),
                                 (wv_sb, wv_in)):
                    for g in range(4):
                        nc.sync.dma_start(out=dst[:, g * 2:(g + 1) * 2, :],
                                          in_=src[:, g * 2:(g + 1) * 2, :])

                # b2 row staged here; broadcast matmuls run in phase 3
                b2row_bf = env["persist"].tile([1, D], bf16)
                b2row = b2t.tile([1, D], f32)
                nc.sync.dma_start(out=b2row,
                                  in_=b2_in.rearrange("(o d) -> o d", o=1))
                nc.vector.tensor_copy(b2row_bf, b2row)

                def ln_block(sc, x_pre=None):
                    if x_pre is not None:
                        x_t = x_pre
                    else:
                        x_t = xp.tile([P, D], f32, tag="x")
                        nc.sync.dma_start(
                            out=x_t, in_=x_in[sc * P:(sc + 1) * P, :])
                    stats = st.tile([P, 2, 6], f32, tag="stats")
                    nc.vector.bn_stats(out=stats[:, 0, :], in_=x_t[:, 0:512])
                    nc.vector.bn_stats(out=stats[:, 1, :], in_=x_t[:, 512:1024])
                    mv = st.tile([P, 2], f32, tag="mv")
                    nc.vector.bn_aggr(out=mv, in_=stats)
                    std = st.tile([P, 1], f32, tag="std")
                    nc.scalar.activation(out=std, in_=mv[:, 1:2],
                                         func=AF.Sqrt, bias=eps_p)
                    rstd = st.tile([P, 1], f32, tag="rstd")
                    nc.vector.reciprocal(out=rstd, in_=std)
                    hbf = hbp.tile([P, D], bf16, tag="h")
                    nc.vector.tensor_scalar(out=hbf, in0=x_t,
                                            scalar1=mv[:, 0:1], scalar2=rstd,
                                            op0=ALU.subtract, op1=ALU.mult)
                    for dc in range(DC):
                        tp = tps.tile([P, P], bf16, tag="tp")
                        nc.tensor.transpose(
                            tp, hbf[:, dc * P:(dc + 1) * P], ident_bf)
                        nc.vector.tensor_scalar(
                            out=hT[:, dc, sc * P:(sc + 1) * P], in0=tp,
                            scalar1=ln1g[:, dc:dc + 1],
                            scalar2=ln1b[:, dc:dc + 1],
                            op0=ALU.mult, op1=ALU.add)

                # LN runs one s-block ahead of its QKV matmuls
                for sc in range(4):
                    ln_block(sc, x_ts[sc])
                for sb in range(4):
                    if sb < 3:
                        for sc in range(4 * (sb + 1), 4 * (sb + 1) + 4):
                            ln_block(sc)
                    # qT / kT for this s-block (bf16 matmuls, fp8 outputs)
                    for dst, w_sb in ((qT, wq_sb), (kT, wk_sb)):
                        for jc in range(DC):
                            ps = mps.tile([P, 512], f32, tag="ps")
                            for dc in range(DC):
                                nc.tensor.matmul(
                                    ps, w_sb[:, dc, jc * P:(jc + 1) * P],
                                    hT[:, dc, sb * 512:(sb + 1) * 512],
                                    start=(dc == 0), stop=(dc == DC - 1))
                            o = dst[:, jc, sb * 512:(sb + 1) * 512]
                            if jc % 2 == 0:
                                nc.vector.tensor_copy(o, ps)
                            else:
                                nc.scalar.copy(o, ps)
                    # V rows for this s-block
                    for skc in range(4 * sb, 4 * sb + 4):
                        for db in range(2):
                            ps = mps.tile([P, 512], f32, tag="ps")
                            for dc in range(DC):
                                nc.tensor.matmul(
                                    ps, hT[:, dc, skc * P:(skc + 1) * P],
                                    wv_sb[:, dc, db * 512:(db + 1) * 512],
                                    start=(dc == 0), stop=(dc == DC - 1))
                            o = V[:, skc, db * 512:(db + 1) * 512]
                            if (skc + db) % 2 == 0:
                                nc.vector.tensor_copy(o, ps)
                            else:
                                nc.scalar.copy(o, ps)

            # w2 preload: right stack, created now (phase 0-2 pools just
            # released), survives until end of this rep
            w2p = whole.enter_context(
                tc.tile_pool(name="w2p", bufs=1, side="right"))
            w2_sb = w2p.tile([P, HC, D], bf16)
            for g in range(8):
                nc.sync.dma_start(out=w2_sb[:, g * 4:(g + 1) * 4, :],
                                  in_=w2_in[:, g * 4:(g + 1) * 4, :])

            # ---- Phase 3: attention + wo + residual + LN2 stats ----
            with ExitStack() as ph:
                wop = ph.enter_context(tc.tile_pool(name="wop", bufs=1))
                etp = ph.enter_context(tc.tile_pool(name="etp", bufs=1))
                ytp = ph.enter_context(tc.tile_pool(name="ytp", bufs=1))
                rbp = ph.enter_context(tc.tile_pool(name="rbp", bufs=1))
                rcp = ph.enter_context(tc.tile_pool(name="rcp", bufs=2))
                bfp = ph.enter_context(tc.tile_pool(name="bfp", bufs=3))
                sqp = ph.enter_context(tc.tile_pool(name="sqp", bufs=2))
                stp = ph.enter_context(tc.tile_pool(name="stp", bufs=1))
                sps = ph.enter_context(
                    tc.tile_pool(name="sps", bufs=2, space="PSUM"))
                ups = ph.enter_context(
                    tc.tile_pool(name="ups", bufs=3, space="PSUM"))
                smps = ph.enter_context(
                    tc.tile_pool(name="smps", bufs=1, space="PSUM"))
                lps = ph.enter_context(
                    tc.tile_pool(name="lps", bufs=2, space="PSUM"))

                wo_sb = wop.tile([P, DC, D], f8)
                for g in range(2):
                    nc.sync.dma_start(out=wo_sb[:, g * 4:(g + 1) * 4, :],
                                      in_=wo_in[:, g * 4:(g + 1) * 4, :])

                # b2 broadcast (deps long ready; PE has slack here)
                for half in range(2):
                    bps = smps.tile([P, QB], f32, tag="sums")
                    nc.tensor.matmul(
                        bps, ones_row,
                        b2row_bf[:, half * 512:(half + 1) * 512],
                        start=True, stop=True)
                    nc.vector.tensor_copy(
                        b2bc[:, half * 512:(half + 1) * 512], bps)

                for qb in range(NQB):
                    q0 = qb * QB
                    ET = etp.tile([P, SC, QB], f8, tag="ET")
                    # scores + softmax sums, sums lagged one skc behind the
                    # exp producing their input (all-ones stationary => the
                    # sum lands pre-broadcast on every partition)
                    sum_ps = smps.tile([P, QB], f32, tag="sums")

                    def emit_score_sum(sp):
                        if USE_DR:
                            if sp % 2 == 1:
                                nc.tensor.matmul(
                                    sum_ps, allones8,
                                    ET[:, sp - 1:sp + 1, :],
                                    start=(sp == 1), stop=(sp == SC - 1),
                                    perf_mode=DR)
                        else:
                            nc.tensor.matmul(sum_ps, allones8_2,
                                             ET[:, sp, :],
                                             start=(sp == 0),
                                             stop=(sp == SC - 1))

                    for skc in range(SC):
                        ps = sps.tile([P, QB], f32, tag="st")
                        if USE_DR:
                            for jp in range(DC // 2):
                                nc.tensor.matmul(
                                    ps,
                                    kT[:, 2 * jp:2 * jp + 2,
                                       skc * P:(skc + 1) * P],
                                    qT[:, 2 * jp:2 * jp + 2, q0:q0 + QB],
                                    start=(jp == 0),
                                    stop=(jp == DC // 2 - 1),
                                    perf_mode=DR)
                        else:
                            for jc in range(DC):
                                nc.tensor.matmul(
                                    ps, kT[:, jc, skc * P:(skc + 1) * P],
                                    qT[:, jc, q0:q0 + QB],
                                    start=(jc == 0), stop=(jc == DC - 1))
                        nc.scalar.activation(out=ET[:, skc, :], in_=ps,
                                             func=AF.Exp, scale=SM_SCALE,
                                             bias=expoff)
                        if skc > 0:
                            emit_score_sum(skc - 1)
                    emit_score_sum(SC - 1)
                    # cheap [1,512] reciprocal of one (pre-broadcast) row;
                    # re-broadcast on the PE after the first UT tile so the
                    # PE never waits on the vector engine
                    recip_bf = rcp.tile([1, QB], bf16, tag="recipbf")
                    with nc.allow_low_precision(
                            reason="softmax 1/sum as bf16 bcast scale"):
                        nc.vector.reciprocal(out=recip_bf,
                                             in_=sum_ps[0:1, :])
                    Rb = rbp.tile([P, QB], f32, tag="Rb")
                    # UT = V.T @ ET, normalized by Rb -> yTs (fp8)
                    yTs = ytp.tile([P, DC, QB], f8, tag="yTs")
                    ps0 = None
                    for dvc in range(DC):
                        if dvc == 1:
                            rb_ps = smps.tile([P, QB], f32, tag="sums")
                            nc.tensor.matmul(rb_ps, ones_row, recip_bf,
                                             start=True, stop=True)
                            nc.vector.tensor_copy(Rb, rb_ps)
                            nc.vector.tensor_tensor(out=yTs[:, 0, :],
                                                    in0=ps0, in1=Rb,
                                                    op=ALU.mult)
                        ps = ups.tile([P, QB], f32, tag="ps")
                        if USE_DR:
                            for sp in range(SC // 2):
                                nc.tensor.matmul(
                                    ps,
                                    V[:, 2 * sp:2 * sp + 2,
                                      dvc * P:(dvc + 1) * P],
                                    ET[:, 2 * sp:2 * sp + 2, :],
                                    start=(sp == 0),
                                    stop=(sp == SC // 2 - 1),
                                    perf_mode=DR)
                        else:
                            for sk in range(SC):
                                nc.tensor.matmul(
                                    ps, V[:, sk, dvc * P:(dvc + 1) * P],
                                    ET[:, sk, :],
                                    start=(sk == 0), stop=(sk == SC - 1))
                        if dvc == 0:
                            ps0 = ps
                        else:
                            nc.vector.tensor_tensor(out=yTs[:, dvc, :],
                                                    in0=ps, in1=Rb,
                                                    op=ALU.mult)
                    # x2T = hT + wo.T @ yTs (bf16), + LN2 partition sums
                    # (lagged one dc so the PE never waits on vector/scalar)
                    bsum_ps = lps.tile([P, QB], f32, tag="lp")
                    sq_ps = lps.tile([P, QB], f32, tag="lp")
                    bts = []
                    sqs = []

                    def emit_stat_sums(d):
                        nc.tensor.matmul(bsum_ps, allones_bf, bts[d],
                                         start=(d == 0), stop=(d == DC - 1))
                        if USE_DR:
                            if d % 2 == 1:
                                nc.tensor.matmul(
                                    sq_ps, allones8, sqs[d - 1],
                                    start=(d == 1), stop=(d == DC - 1),
                                    perf_mode=DR)
                        else:
                            nc.tensor.matmul(sq_ps, allones8_2,
                                             sqs[d][:, d % 2, :],
                                             start=(d == 0),
                                             stop=(d == DC - 1))

                    for dc in range(DC):
                        ps = ups.tile([P, QB], f32, tag="ps")
                        if USE_DR:
                            for vp in range(DC // 2):
                                nc.tensor.matmul(
                                    ps,
                                    wo_sb[:, 2 * vp:2 * vp + 2,
                                          dc * P:(dc + 1) * P],
                                    yTs[:, 2 * vp:2 * vp + 2, :],
                                    start=(vp == 0),
                                    stop=(vp == DC // 2 - 1),
                                    perf_mode=DR)
                        else:
                            for vc in range(DC):
                                nc.tensor.matmul(
                                    ps, wo_sb[:, vc, dc * P:(dc + 1) * P],
                                    yTs[:, vc, :],
                                    start=(vc == 0), stop=(vc == DC - 1))
                        if qb == 0:
                            bt = x2b0[:, dc, :]
                        else:
                            bt = bfp.tile([P, QB], bf16, tag="btw")
                        nc.vector.tensor_tensor(out=bt, in0=ps,
                                                in1=hT[:, dc, q0:q0 + QB],
                                                op=ALU.add)
                        if qb > 0:
                            nc.sync.dma_start(
                                out=x2T_dram[:, dc, q0:q0 + QB], in_=bt)
                        if dc % 2 == 0:
                            sq_pair = sqp.tile([P, 2, QB], f8, tag="sq")
                        nc.scalar.activation(out=sq_pair[:, dc % 2, :],
                                             in_=bt, func=AF.Square)
                        bts.append(bt)
                        if dc % 2 == 1:
                            sqs.extend([sq_pair, sq_pair])
                        if dc > 0:
                            emit_stat_sums(dc - 1)
                    emit_stat_sums(DC - 1)
                    # LN2 stats (full-width rows; no broadcasts needed)
                    mu = stp.tile([P, QB], f32, tag="mu")
                    nc.scalar.activation(out=mu, in_=bsum_ps, func=AF.Copy,
                                         scale=1.0 / D)
                    msq = stp.tile([P, QB], f32, tag="msq")
                    nc.scalar.activation(out=msq, in_=sq_ps, func=AF.Copy,
                                         scale=1.0 / D)
                    var = stp.tile([P, QB], f32, tag="var")
                    nc.vector.tensor_tensor(out=var, in0=mu, in1=mu,
                                            op=ALU.mult)
                    nc.vector.tensor_tensor(out=var, in0=msq, in1=var,
                                            op=ALU.subtract)
                    stdv = stp.tile([P, QB], f32, tag="stdv")
                    nc.scalar.activation(out=stdv, in_=var, func=AF.Sqrt,
                                         bias=eps_p)
                    with nc.allow_low_precision(
                            reason="LN2 1/std as bf16 scale row"):
                        nc.vector.reciprocal(out=rsab[:, 2 * qb, :],
                                             in_=stdv)
                    nc.vector.tensor_tensor(out=rsab[:, 2 * qb + 1, :],
                                            in0=mu, in1=rsab[:, 2 * qb, :],
                                            op=ALU.mult)

        # ---- Phase 4/5: folded-LN2 MLP, natural-layout output ----
        with ExitStack() as ph:
            w1p = ph.enter_context(tc.tile_pool(name="w1p", bufs=6))
            x2bp = ph.enter_context(tc.tile_pool(name="x2bp", bufs=2))
            gtt = ph.enter_context(tc.tile_pool(name="gtt", bufs=3))
            gtp = ph.enter_context(tc.tile_pool(name="gtp", bufs=1))
            otp = ph.enter_context(tc.tile_pool(name="otp", bufs=3))
            gps = ph.enter_context(
                tc.tile_pool(name="gps", bufs=3, space="PSUM"))
            mps2 = ph.enter_context(
                tc.tile_pool(name="mps2", bufs=3, space="PSUM"))

            for mb in range(NMB):
                s0 = mb * MB
                if mb == 0:
                    btmb = x2b0
                else:
                    btmb = x2bp.tile([P, DC, MB], bf16, tag="x2b")
                    for g in range(2):
                        nc.sync.dma_start(
                            out=btmb[:, g * 4:(g + 1) * 4, :],
                            in_=x2T_dram[:, g * 4:(g + 1) * 4, s0:s0 + MB])
                # GT = gelu(rstd*(w1g2.T @ x2T) + (mu*rstd)*w1gs + b1_eff)
                GTb = gtp.tile([P, HC, MB], bf16, tag="GTb")
                for hc in range(HC):
                    w1t = w1p.tile([P, DC, P], bf16, tag="w1t")
                    nc.sync.dma_start(out=w1t, in_=w1_in[hc])
                    ps = gps.tile([P, MB], f32, tag="gt")
                    for dc in range(DC):
                        nc.tensor.matmul(
                            ps, w1t[:, dc, :], btmb[:, dc, :],
                            start=(dc == 0), stop=(dc == DC - 1))
                    t1 = gtt.tile([P, MB], f32, tag="t1")
                    nc.vector.tensor_tensor(out=t1, in0=ps,
                                            in1=rsab[:, 2 * mb, :],
                                            op=ALU.mult)
                    t2 = gtt.tile([P, MB], f32, tag="t1")
                    nc.vector.scalar_tensor_tensor(
                        out=t2, in0=rsab[:, 2 * mb + 1, :],
                        scalar=w1gs[:, hc:hc + 1], in1=t1,
                        op0=ALU.mult, op1=ALU.add)
                    nc.scalar.activation(out=GTb[:, hc, :], in_=t2,
                                         func=AF.Gelu,
                                         bias=b1c[:, hc:hc + 1])
                # out[s, :] = x2[s, :] + GT.T @ w2 + b2  (natural layout)
                for ssc in range(MB // P):
                    for dh in range(2):
                        ps = mps2.tile([P, 512], f32, tag="mo")
                        for hc in range(HC):
                            nc.tensor.matmul(
                                ps, GTb[:, hc, ssc * P:(ssc + 1) * P],
                                w2_sb[:, hc, dh * 512:(dh + 1) * 512],
                                start=(hc == 0), stop=False)
                        for l in range(4):
                            dc = dh * 4 + l
                            nc.tensor.matmul(
                                ps[:, l * P:(l + 1) * P],
                                btmb[:, dc, ssc * P:(ssc + 1) * P],
                                ident_bf,
                                start=False, stop=(l == 3),
                                skip_group_check=True)
                        o1 = otp.tile([P, 512], f32, tag="o1")
                        nc.vector.tensor_tensor(
                            out=o1, in0=ps,
                            in1=b2bc[:, dh * 512:(dh + 1) * 512], op=ALU.add)
                        r0 = s0 + ssc * P
                        nc.sync.dma_start(
                            out=out_dram[r0:r0 + P,
                                         dh * 512:(dh + 1) * 512],
                            in_=o1)


_CACHED = {}


def _get_nc():
    if "nc" not in _CACHED:
        import concourse.bass as bass
        import concourse.mybir as mybir
        import concourse.tile as tile
        from concourse import bacc
        nc = bacc.Bacc()
        _CACHED["nc"] = build(nc, bass, mybir, tile)
    return _CACHED["nc"]


def _bf(a):
    return np.asarray(a, np.float32).astype(ml_dtypes.bfloat16)


def _f8(a):
    return np.clip(np.asarray(a, np.float32), -240.0, 240.0).astype(
        ml_dtypes.float8_e4m3)


def _tile_pcn(w, dtype_fn):
    """[D, N] -> [P, DC_in, N] so each DMA row is contiguous."""
    w = np.asarray(w, np.float32)
    d, n = w.shape
    c = d // P
    return dtype_fn(w.reshape(c, P, n).transpose(1, 0, 2).copy())


def prepare_inputs(inputs):
    x = np.asarray(inputs["x"], dtype=np.float32)

    # LN2 fold: w1_eff = w1 * g2 (per input row), w1gs = -sum_d w1_eff[d,h],
    # b1_eff = b1 + w1.T @ ln2_b
    w1f = np.asarray(inputs["w1"], np.float32)
    g2 = np.asarray(inputs["ln2_g"], np.float32)
    bln2 = np.asarray(inputs["ln2_b"], np.float32)
    w1_eff = w1f * g2[:, None]
    w1gs_neg = -w1_eff.sum(axis=0)
    b1_eff = np.asarray(inputs["b1"], np.float32) + w1f.T @ bln2
    w1t = (_bf(w1_eff).reshape(DC, P, HC, P).transpose(2, 1, 0, 3).copy())
    shared = {
        "wq": _tile_pcn(inputs["wq"], _bf),
        "wk": _tile_pcn(inputs["wk"], _bf),
        "wv": _tile_pcn(inputs["wv"], _bf),
        "wo": _tile_pcn(inputs["wo"], _f8),
        "w1": w1t, "w1gs": w1gs_neg.astype(np.float32),
        "w2": _tile_pcn(inputs["w2"], _bf),
        "ln1_g": np.asarray(inputs["ln1_g"], np.float32),
        "ln1_b": np.asarray(inputs["ln1_b"], np.float32),
        "b1": b1_eff.astype(np.float32),
        "b2": np.asarray(inputs["b2"], np.float32),
    }
    return [dict(shared, x=np.ascontiguousarray(x[i])) for i in range(N_CORES)]


def kernel(**inputs):
    from concourse.bass_utils import run_bass_kernel_spmd

    nc = _get_nc()
    in_maps = prepare_inputs(inputs)
    res = run_bass_kernel_spmd(nc, in_maps, list(range(N_CORES)))
    out = np.stack([res.results[i]["out"] for i in range(N_CORES)], axis=0)
    return out.astype(np.float32)


# revision 44
# speedup vs baseline: 1.0245x; 1.0245x over previous
"""Trainium2 Bass kernel for a dense transformer block (nn_Block_58377195487260).

Reference (per batch element, fp32):
    h   = LN1(x)*g1 + b1ln
    q,k,v = h@wq, h@wk, h@wv
    s   = q@k^T / sqrt(dk);  a = softmax(s);  y = (a@v)@wo
    x2  = h + y
    mlp = gelu(LN2(x2) @ w1 + b1) @ w2 + b2
    out = x2 + mlp

Sharding: data-parallel over batch. B=8 == 8 NeuronCores; core i computes
batch element i end-to-end (no collectives).

Numerics/performance strategy (PE-bound kernel, ~90% tensor occupancy):
  - QKV and the MLP run in bf16 (fp8 there fails the 2e-2 tolerance).
  - The attention core runs fp8-e4m3 with DoubleRow perf mode (2 k-chunks
    per pass): scores (qT8 x kT8), softmax sums (ones8 x ET), A@V (V8 x ET),
    and the wo projection (wo8 x yTs8).  exp is computed with a constant
    offset -2.5 so the unnormalized weights stay < 240 (TRN fp8e4 max);
    the offset cancels in the normalization.
  - LN2 statistics are computed during the attention phase (partition-sum
    matmuls on the freshly produced x2 tiles) so the MLP phase starts with
    rstd/mean broadcasts already in SBUF.
  - The MLP output is produced directly in natural [s, d] layout
    (stationary = GT chunks), with the x2 residual folded into the same
    PSUM accumulation via identity matmuls: no output transposes.
  - x2 spills to DRAM in bf16 (block 0 stays in SBUF) and is prefetched
    back during the MLP phase; w2 is loaded during phase 3 onto the right
    SBUF stack so it is resident when the attention pools release the left.
"""

import numpy as np
import ml_dtypes
from contextlib import ExitStack

P = 128
B, S, D, H = 8, 2048, 1024, 4096
DC = D // P          # 8  d-chunks
HC = H // P          # 32 h-chunks
SC = S // P          # 16 s-chunks
QB = 512             # attention sq-block == ln2/mlp s-block
NQB = S // QB        # 4
MB = 512
NMB = S // MB        # 4
EPS = 1e-5
SM_SCALE = 1.0 / 32.0   # 1/sqrt(1024)
EXP_OFF = -2.5          # exp offset; cancels in softmax normalization
# DoubleRow fp8 matmuls process two k-chunks per pass on the attention
# path.  (The PE clock varies run-to-run between ~2.0 and ~2.4 GHz
# independent of this flag; at equal clock DR saved ~115us.)
USE_DR = True

N_CORES = 8


def build(nc, bass, mybir, tile):
    f32 = mybir.dt.float32
    bf16 = mybir.dt.bfloat16
    f8 = mybir.dt.float8e4

    x_in = nc.declare_dram_parameter("x", [S, D], f32, isOutput=False)
    # pre-tiled [partition, in_chunk, out_col] so each load is contiguous
    wq_in = nc.declare_dram_parameter("wq", [P, DC, D], bf16, isOutput=False)
    wk_in = nc.declare_dram_parameter("wk", [P, DC, D], bf16, isOutput=False)
    wv_in = nc.declare_dram_parameter("wv", [P, DC, D], bf16, isOutput=False)
    wo_in = nc.declare_dram_parameter("wo", [P, DC, D], f8, isOutput=False)
    # w1 pre-scaled by ln2_g, tiled [hc][p][dc][n]; w1gs = -sum_d w1g2[d,h];
    # b1 arrives as b1 + w1.T @ ln2_b (LN2 folded into the GT matmul)
    w1_in = nc.declare_dram_parameter("w1", [HC, P, DC, P], bf16, isOutput=False)
    w1gs_in = nc.declare_dram_parameter("w1gs", [H], f32, isOutput=False)
    w2_in = nc.declare_dram_parameter("w2", [P, HC, D], bf16, isOutput=False)
    ln1g_in = nc.declare_dram_parameter("ln1_g", [D], f32, isOutput=False)
    ln1b_in = nc.declare_dram_parameter("ln1_b", [D], f32, isOutput=False)
    b1_in = nc.declare_dram_parameter("b1", [H], f32, isOutput=False)
    b2_in = nc.declare_dram_parameter("b2", [D], f32, isOutput=False)
    out_dram = nc.declare_dram_parameter("out", [S, D], f32, isOutput=True)

    from concourse.masks import make_identity

    with tile.TileContext(nc) as tc, ExitStack() as top:
        const = top.enter_context(tc.tile_pool(name="const", bufs=1))
        persist = top.enter_context(
            tc.tile_pool(name="persist", bufs=1, side="right"))
        dram = top.enter_context(tc.tile_pool(name="dram", bufs=1, space="DRAM"))

        ident_bf = const.tile([P, P], bf16)
        make_identity(nc, ident_bf)
        eps_p = const.tile([P, 1], f32)
        nc.vector.memset(eps_p, EPS)
        eps_1 = const.tile([1, 1], f32)
        nc.vector.memset(eps_1, EPS)
        ones_bf = const.tile([P, 1], bf16)
        nc.vector.memset(ones_bf, 1.0)
        expoff = const.tile([P, 1], f32)
        nc.vector.memset(expoff, EXP_OFF)
        # all-ones stationaries: partition sums land on every partition
        # (pre-broadcast), full-width M=128 so no FWL/M=1 penalties
        allones_bf = const.tile([P, P], bf16)
        nc.vector.memset(allones_bf, 1.0)
        allones8_t = const.tile([P, 2, P], f8)
        nc.vector.memset(allones8_t, 1.0)
        allones8 = allones8_t
        allones8_2 = allones8_t[:, 0, :]
        ones_row = const.tile([1, P], bf16)
        nc.vector.memset(ones_row, 1.0)

        # per-partition views of gains/biases: [P, nchunk], column c = chunk c
        ln1g = const.tile([P, DC], f32)
        ln1b = const.tile([P, DC], f32)
        b1c = const.tile([P, HC], f32)
        w1gs = const.tile([P, HC], f32)
        for dst, src in ((ln1g, ln1g_in), (ln1b, ln1b_in),
                         (b1c, b1_in), (w1gs, w1gs_in)):
            nc.sync.dma_start(out=dst, in_=src.rearrange("(c p) -> p c", p=P))

        # persistent across phase 3 -> 4/5
        x2b0 = persist.tile([P, DC, MB], bf16)     # x2 block 0 (stays on-chip)
        rsab = persist.tile([P, 2 * NMB, MB], bf16)  # rstd / mu*rstd bcast
        b2bc = persist.tile([P, D], bf16)          # b2 broadcast over partitions

        x2T_dram = dram.tile([P, DC, S], bf16)     # x2 spill, blocks 1..3

        import os
        for _rep in range(int(os.environ.get("BENCH_REPS", "1"))):
            _build_body(nc, tc, mybir, locals())

    nc.finalize()
    return nc


def _build_body(nc, tc, mybir, env):
    f32 = mybir.dt.float32
    bf16 = mybir.dt.bfloat16
    f8 = mybir.dt.float8e4
    AF = mybir.ActivationFunctionType
    ALU = mybir.AluOpType
    DR = mybir.MatmulPerfMode.DoubleRow
    (x_in, wq_in, wk_in, wv_in, wo_in, w1_in, w2_in, b2_in, out_dram,
     x2T_dram, ident_bf, eps_p, eps_1, ones_bf, ones_row,
     ln1g, ln1b, b1c, w1gs, x2b0, rsab, b2bc, expoff,
     allones_bf, allones8, allones8_2) = (
        env["x_in"], env["wq_in"], env["wk_in"], env["wv_in"], env["wo_in"],
        env["w1_in"], env["w2_in"], env["b2_in"], env["out_dram"],
        env["x2T_dram"], env["ident_bf"], env["eps_p"], env["eps_1"],
        env["ones_bf"], env["ones_row"], env["ln1g"],
        env["ln1b"], env["b1c"], env["w1gs"], env["x2b0"], env["rsab"],
        env["b2bc"], env["expoff"],
        env["allones_bf"], env["allones8"], env["allones8_2"])

    with ExitStack() as whole:
        w2_sb = None
        with ExitStack() as attn:
            act = attn.enter_context(tc.tile_pool(name="act", bufs=1))
            hT = act.tile([P, DC, S], bf16)          # [d, s] post-LN1
            qT = act.tile([P, DC, S], f8)            # [dk, s]
            kT = act.tile([P, DC, S], f8)
            V = act.tile([P, SC, D], f8)             # [s, dv]

            # ---- Phase 0-2 per 512-col s-block: LN1 + transpose + QKV ----
            with ExitStack() as ph:
                wp = ph.enter_context(tc.tile_pool(name="wp", bufs=1))
                xp = ph.enter_context(tc.tile_pool(name="xp", bufs=3))
                hbp = ph.enter_context(tc.tile_pool(name="hbp", bufs=3))
                st = ph.enter_context(tc.tile_pool(name="st", bufs=4))
                b2t = ph.enter_context(tc.tile_pool(name="b2t", bufs=1))
                tps = ph.enter_context(
                    tc.tile_pool(name="tps", bufs=4, space="PSUM"))
                mps = ph.enter_context(
                    tc.tile_pool(name="mps", bufs=4, space="PSUM"))

                # first x rows start streaming before the big weight loads
                x_ts = []
                for sc in range(4):
                    x_t = xp.tile([P, D], f32, tag="x")
                    nc.sync.dma_start(out=x_t,
                                      in_=x_in[sc * P:(sc + 1) * P, :])
                    x_ts.append(x_t)

                wq_sb = wp.tile([P, DC, D], bf16)
                wk_sb = wp.tile([P, DC, D], bf16)
                wv_sb = wp.tile([P, DC, D], bf16)
                for dst, src in ((wq_sb, wq_in), (wk_sb, wk_in),
                                 (wv_sb, wv_in)):
                    for g in range(4):
                        nc.sync.dma_start(out=dst[:, g * 2:(g + 1) * 2, :],
                                          in_=src[:, g * 2:(g + 1) * 2, :])

                # b2 row staged here; broadcast matmuls run in phase 3
                b2row_bf = env["persist"].tile([1, D], bf16)
                b2row = b2t.tile([1, D], f32)
                nc.sync.dma_start(out=b2row,
                                  in_=b2_in.rearrange("(o d) -> o d", o=1))
                nc.vector.tensor_copy(b2row_bf, b2row)

                def ln_block(sc, x_pre=None):
                    if x_pre is not None:
                        x_t = x_pre
                    else:
                        x_t = xp.tile([P, D], f32, tag="x")
                        nc.sync.dma_start(
                            out=x_t, in_=x_in[sc * P:(sc + 1) * P, :])
                    stats = st.tile([P, 2, 6], f32, tag="stats")
                    nc.vector.bn_stats(out=stats[:, 0, :], in_=x_t[:, 0:512])
                    nc.vector.bn_stats(out=stats[:, 1, :], in_=x_t[:, 512:1024])
                    mv = st.tile([P, 2], f32, tag="mv")
                    nc.vector.bn_aggr(out=mv, in_=stats)
                    std = st.tile([P, 1], f32, tag="std")
                    nc.scalar.activation(out=std, in_=mv[:, 1:2],
                                         func=AF.Sqrt, bias=eps_p)
                    rstd = st.tile([P, 1], f32, tag="rstd")
                    nc.vector.reciprocal(out=rstd, in_=std)
                    hbf = hbp.tile([P, D], bf16, tag="h")
                    nc.vector.tensor_scalar(out=hbf, in0=x_t,
                                            scalar1=mv[:, 0:1], scalar2=rstd,
                                            op0=ALU.subtract, op1=ALU.mult)
                    for dc in range(DC):
                        tp = tps.tile([P, P], bf16, tag="tp")
                        nc.tensor.transpose(
                            tp, hbf[:, dc * P:(dc + 1) * P], ident_bf)
                        nc.vector.tensor_scalar(
                            out=hT[:, dc, sc * P:(sc + 1) * P], in0=tp,
                            scalar1=ln1g[:, dc:dc + 1],
                            scalar2=ln1b[:, dc:dc + 1],
                            op0=ALU.mult, op1=ALU.add)

                # LN runs one s-block ahead of its QKV matmuls
                for sc in range(4):
                    ln_block(sc, x_ts[sc])
                for sb in range(4):
                    if sb < 3:
                        for sc in range(4 * (sb + 1), 4 * (sb + 1) + 4):
                            ln_block(sc)
                    # qT / kT for this s-block (bf16 matmuls, fp8 outputs)
                    for dst, w_sb in ((qT, wq_sb), (kT, wk_sb)):
                        for jc in range(DC):
                            ps = mps.tile([P, 512], f32, tag="ps")
                            for dc in range(DC):
                                nc.tensor.matmul(
                                    ps, w_sb[:, dc, jc * P:(jc + 1) * P],
                                    hT[:, dc, sb * 512:(sb + 1) * 512],
                                    start=(dc == 0), stop=(dc == DC - 1))
                            o = dst[:, jc, sb * 512:(sb + 1) * 512]
                            if jc % 2 == 0:
                                nc.vector.tensor_copy(o, ps)
                            else:
                                nc.scalar.copy(o, ps)
                    # V rows for this s-block
                    for skc in range(4 * sb, 4 * sb + 4):
                        for db in range(2):
                            ps = mps.tile([P, 512], f32, tag="ps")
                            for dc in range(DC):
                                nc.tensor.matmul(
                                    ps, hT[:, dc, skc * P:(skc + 1) * P],
                                    wv_sb[:, dc, db * 512:(db + 1) * 512],
                                    start=(dc == 0), stop=(dc == DC - 1))
                            o = V[:, skc, db * 512:(db + 1) * 512]
                            if (skc + db) % 2 == 0:
                                nc.vector.tensor_copy(o, ps)
                            else:
                                nc.scalar.copy(o, ps)

            # w2 preload: right stack, created now (phase 0-2 pools just
            # released), survives until end of this rep
            w2p = whole.enter_context(
                tc.tile_pool(name="w2p", bufs=1, side="right"))
            w2_sb = w2p.tile([P, HC, D], bf16)
            for g in range(8):
                nc.sync.dma_start(out=w2_sb[:, g * 4:(g + 1) * 4, :],
                                  in_=w2_in[:, g * 4:(g + 1) * 4, :])

            # ---- Phase 3: attention + wo + residual + LN2 stats ----
            with ExitStack() as ph:
                wop = ph.enter_context(tc.tile_pool(name="wop", bufs=1))
                etp = ph.enter_context(tc.tile_pool(name="etp", bufs=1))
                ytp = ph.enter_context(tc.tile_pool(name="ytp", bufs=1))
                rbp = ph.enter_context(tc.tile_pool(name="rbp", bufs=1))
                rcp = ph.enter_context(tc.tile_pool(name="rcp", bufs=2))
                bfp = ph.enter_context(tc.tile_pool(name="bfp", bufs=3))
                sqp = ph.enter_context(tc.tile_pool(name="sqp", bufs=2))
                stp = ph.enter_context(tc.tile_pool(name="stp", bufs=1))
                sps = ph.enter_context(
                    tc.tile_pool(name="sps", bufs=2, space="PSUM"))
                ups = ph.enter_context(
                    tc.tile_pool(name="ups", bufs=3, space="PSUM"))
                smps = ph.enter_context(
                    tc.tile_pool(name="smps", bufs=1, space="PSUM"))
                lps = ph.enter_context(
                    tc.tile_pool(name="lps", bufs=2, space="PSUM"))

                wo_sb = wop.tile([P, DC, D], f8)
                for g in range(2):
                    nc.sync.dma_start(out=wo_sb[:, g * 4:(g + 1) * 4, :],
                                      in_=wo_in[:, g * 4:(g + 1) * 4, :])

                # b2 broadcast (deps long ready; PE has slack here)
                for half in range(2):
                    bps = smps.tile([P, QB], f32, tag="sums")
                    nc.tensor.matmul(
                        bps, ones_row,
                        b2row_bf[:, half * 512:(half + 1) * 512],
                        start=True, stop=True)
                    nc.vector.tensor_copy(
                        b2bc[:, half * 512:(half + 1) * 512], bps)

                for qb in range(NQB):
                    q0 = qb * QB
                    ET = etp.tile([P, SC, QB], f8, tag="ET")
                    # scores + softmax sums, sums lagged one skc behind the
                    # exp producing their input (all-ones stationary => the
                    # sum lands pre-broadcast on every partition)
                    sum_ps = smps.tile([P, QB], f32, tag="sums")

                    def emit_score_sum(sp):
                        if USE_DR:
                            if sp % 2 == 1:
                                nc.tensor.matmul(
                                    sum_ps, allones8,
                                    ET[:, sp - 1:sp + 1, :],
                                    start=(sp == 1), stop=(sp == SC - 1),
                                    perf_mode=DR)
                        else:
                            nc.tensor.matmul(sum_ps, allones8_2,
                                             ET[:, sp, :],
                                             start=(sp == 0),
                                             stop=(sp == SC - 1))

                    for skc in range(SC):
                        ps = sps.tile([P, QB], f32, tag="st")
                        if USE_DR:
                            for jp in range(DC // 2):
                                nc.tensor.matmul(
                                    ps,
                                    kT[:, 2 * jp:2 * jp + 2,
                                       skc * P:(skc + 1) * P],
                                    qT[:, 2 * jp:2 * jp + 2, q0:q0 + QB],
                                    start=(jp == 0),
                                    stop=(jp == DC // 2 - 1),
                                    perf_mode=DR)
                        else:
                            for jc in range(DC):
                                nc.tensor.matmul(
                                    ps, kT[:, jc, skc * P:(skc + 1) * P],
                                    qT[:, jc, q0:q0 + QB],
                                    start=(jc == 0), stop=(jc == DC - 1))
                        nc.scalar.activation(out=ET[:, skc, :], in_=ps,
                                             func=AF.Exp, scale=SM_SCALE,
                                             bias=expoff)
                        if skc > 0:
                            emit_score_sum(skc - 1)
                    emit_score_sum(SC - 1)
                    # cheap [1,512] reciprocal of one (pre-broadcast) row;
                    # re-broadcast on the PE after the first UT tile so the
                    # PE never waits on the vector engine
                    recip_bf = rcp.tile([1, QB], bf16, tag="recipbf")
                    with nc.allow_low_precision(
                            reason="softmax 1/sum as bf16 bcast scale"):
                        nc.vector.reciprocal(out=recip_bf,
                                             in_=sum_ps[0:1, :])
                    Rb = rbp.tile([P, QB], f32, tag="Rb")
                    # UT = V.T @ ET, normalized by Rb -> yTs (fp8)
                    yTs = ytp.tile([P, DC, QB], f8, tag="yTs")
                    ps0 = None
                    for dvc in range(DC):
                        if dvc == 1:
                            rb_ps = smps.tile([P, QB], f32, tag="sums")
                            nc.tensor.matmul(rb_ps, ones_row, recip_bf,
                                             start=True, stop=True)
                            nc.vector.tensor_copy(Rb, rb_ps)
                            nc.vector.tensor_tensor(out=yTs[:, 0, :],
                                                    in0=ps0, in1=Rb,
                                                    op=ALU.mult)
                        ps = ups.tile([P, QB], f32, tag="ps")
                        if USE_DR:
                            for sp in range(SC // 2):
                                nc.tensor.matmul(
                                    ps,
                                    V[:, 2 * sp:2 * sp + 2,
                                      dvc * P:(dvc + 1) * P],
                                    ET[:, 2 * sp:2 * sp + 2, :],
                                    start=(sp == 0),
                                    stop=(sp == SC // 2 - 1),
                                    perf_mode=DR)
                        else:
                            for sk in range(SC):
                                nc.tensor.matmul(
                                    ps, V[:, sk, dvc * P:(dvc + 1) * P],
                                    ET[:, sk, :],
                                    start=(sk == 0), stop=(sk == SC - 1))
                        if dvc == 0:
                            ps0 = ps
                        else:
                            nc.vector.tensor_tensor(out=yTs[:, dvc, :],
                                                    in0=ps, in1=Rb,
                                                    op=ALU.mult)
                    # x2T = hT + wo.T @ yTs (bf16), + LN2 partition sums
                    # (lagged one dc so the PE never waits on vector/scalar)
                    bsum_ps = lps.tile([P, QB], f32, tag="lp")
                    sq_ps = lps.tile([P, QB], f32, tag="lp")
                    bts = []
                    sqs = []

                    def emit_stat_sums(d):
                        nc.tensor.matmul(bsum_ps, allones_bf, bts[d],
                                         start=(d == 0), stop=(d == DC - 1))
                        if USE_DR:
                            if d % 2 == 1:
                                nc.tensor.matmul(
                                    sq_ps, allones8, sqs[d - 1],
                                    start=(d == 1), stop=(d == DC - 1),
                                    perf_mode=DR)
                        else:
                            nc.tensor.matmul(sq_ps, allones8_2,
                                             sqs[d][:, d % 2, :],
                                             start=(d == 0),
                                             stop=(d == DC - 1))

                    for dc in range(DC):
                        ps = ups.tile([P, QB], f32, tag="ps")
                        if USE_DR:
                            for vp in range(DC // 2):
                                nc.tensor.matmul(
                                    ps,
                                    wo_sb[:, 2 * vp:2 * vp + 2,
                                          dc * P:(dc + 1) * P],
                                    yTs[:, 2 * vp:2 * vp + 2, :],
                                    start=(vp == 0),
                                    stop=(vp == DC // 2 - 1),
                                    perf_mode=DR)
                        else:
                            for vc in range(DC):
                                nc.tensor.matmul(
                                    ps, wo_sb[:, vc, dc * P:(dc + 1) * P],
                                    yTs[:, vc, :],
                                    start=(vc == 0), stop=(vc == DC - 1))
                        if qb == 0:
                            bt = x2b0[:, dc, :]
                        else:
                            bt = bfp.tile([P, QB], bf16, tag="btw")
                        nc.vector.tensor_tensor(out=bt, in0=ps,
                                                in1=hT[:, dc, q0:q0 + QB],
                                                op=ALU.add)
                        if qb > 0:
                            nc.sync.dma_start(
                                out=x2T_dram[:, dc, q0:q0 + QB], in_=bt)
                        if dc % 2 == 0:
                            sq_pair = sqp.tile([P, 2, QB], f8, tag="sq")
                        nc.vector.tensor_tensor(out=sq_pair[:, dc % 2, :],
                                                in0=bt, in1=bt, op=ALU.mult)
                        bts.append(bt)
                        if dc % 2 == 1:
                            sqs.extend([sq_pair, sq_pair])
                        if dc > 0:
                            emit_stat_sums(dc - 1)
                    emit_stat_sums(DC - 1)
                    # LN2 stats (full-width rows; no broadcasts needed)
                    mu = stp.tile([P, QB], f32, tag="mu")
                    nc.scalar.activation(out=mu, in_=bsum_ps, func=AF.Copy,
                                         scale=1.0 / D)
                    msq = stp.tile([P, QB], f32, tag="msq")
                    nc.scalar.activation(out=msq, in_=sq_ps, func=AF.Copy,
                                         scale=1.0 / D)
                    var = stp.tile([P, QB], f32, tag="var")
                    nc.vector.tensor_tensor(out=var, in0=mu, in1=mu,
                                            op=ALU.mult)
                    nc.vector.tensor_tensor(out=var, in0=msq, in1=var,
                                            op=ALU.subtract)
                    stdv = stp.tile([P, QB], f32, tag="stdv")
                    nc.scalar.activation(out=stdv, in_=var, func=AF.Sqrt,
                                         bias=eps_p)
                    with nc.allow_low_precision(
                            reason="LN2 1/std as bf16 scale row"):
                        nc.vector.reciprocal(out=rsab[:, 2 * qb, :],
                                             in_=stdv)
                    nc.vector.tensor_tensor(out=rsab[:, 2 * qb + 1, :],
                                            in0=mu, in1=rsab[:, 2 * qb, :],
                                            op=ALU.mult)

        # ---- Phase 4/5: folded-LN2 MLP, natural-layout output ----
        with ExitStack() as ph:
            w1p = ph.enter_context(tc.tile_pool(name="w1p", bufs=6))
            x2bp = ph.enter_context(tc.tile_pool(name="x2bp", bufs=2))
            gtt = ph.enter_context(tc.tile_pool(name="gtt", bufs=3))
            gtp = ph.enter_context(tc.tile_pool(name="gtp", bufs=1))
            otp = ph.enter_context(tc.tile_pool(name="otp", bufs=3))
            gps = ph.enter_context(
                tc.tile_pool(name="gps", bufs=3, space="PSUM"))
            mps2 = ph.enter_context(
                tc.tile_pool(name="mps2", bufs=3, space="PSUM"))

            for mb in range(NMB):
                s0 = mb * MB
                if mb == 0:
                    btmb = x2b0
                else:
                    btmb = x2bp.tile([P, DC, MB], bf16, tag="x2b")
                    for g in range(2):
                        nc.sync.dma_start(
                            out=btmb[:, g * 4:(g + 1) * 4, :],
                            in_=x2T_dram[:, g * 4:(g + 1) * 4, s0:s0 + MB])
                # GT = gelu(rstd*(w1g2.T @ x2T) + (mu*rstd)*w1gs + b1_eff)
                GTb = gtp.tile([P, HC, MB], bf16, tag="GTb")
                for hc in range(HC):
                    w1t = w1p.tile([P, DC, P], bf16, tag="w1t")
                    nc.sync.dma_start(out=w1t, in_=w1_in[hc])
                    ps = gps.tile([P, MB], f32, tag="gt")
                    for dc in range(DC):
                        nc.tensor.matmul(
                            ps, w1t[:, dc, :], btmb[:, dc, :],
                            start=(dc == 0), stop=(dc == DC - 1))
                    t1 = gtt.tile([P, MB], f32, tag="t1")
                    nc.vector.tensor_tensor(out=t1, in0=ps,
                                            in1=rsab[:, 2 * mb, :],
                                            op=ALU.mult)
                    t2 = gtt.tile([P, MB], f32, tag="t1")
                    nc.vector.scalar_tensor_tensor(
                        out=t2, in0=rsab[:, 2 * mb + 1, :],
                        scalar=w1gs[:, hc:hc + 1], in1=t1,
                        op0=ALU.mult, op1=ALU.add)
                    nc.scalar.activation(out=GTb[:, hc, :], in_=t2,
                                         func=AF.Gelu,
                                         bias=b1c[:, hc:hc + 1])
                # out[s, :] = x2[s, :] + GT.T @ w2 + b2  (natural layout)
                for ssc in range(MB // P):
                    for dh in range(2):
                        ps = mps2.tile([P, 512], f32, tag="mo")
                        for hc in range(HC):
                            nc.tensor.matmul(
                                ps, GTb[:, hc, ssc * P:(ssc + 1) * P],
                                w2_sb[:, hc, dh * 512:(dh + 1) * 512],
                                start=(hc == 0), stop=False)
                        for l in range(4):
                            dc = dh * 4 + l
                            nc.tensor.matmul(
                                ps[:, l * P:(l + 1) * P],
                                btmb[:, dc, ssc * P:(ssc + 1) * P],
                                ident_bf,
                                start=False, stop=(l == 3),
                                skip_group_check=True)
                        o1 = otp.tile([P, 512], f32, tag="o1")
                        nc.vector.tensor_tensor(
                            out=o1, in0=ps,
                            in1=b2bc[:, dh * 512:(dh + 1) * 512], op=ALU.add)
                        r0 = s0 + ssc * P
                        nc.sync.dma_start(
                            out=out_dram[r0:r0 + P,
                                         dh * 512:(dh + 1) * 512],
                            in_=o1)


_CACHED = {}


def _get_nc():
    if "nc" not in _CACHED:
        import concourse.bass as bass
        import concourse.mybir as mybir
        import concourse.tile as tile
        from concourse import bacc
        nc = bacc.Bacc()
        _CACHED["nc"] = build(nc, bass, mybir, tile)
    return _CACHED["nc"]


def _bf(a):
    return np.asarray(a, np.float32).astype(ml_dtypes.bfloat16)


def _f8(a):
    return np.clip(np.asarray(a, np.float32), -240.0, 240.0).astype(
        ml_dtypes.float8_e4m3)


def _tile_pcn(w, dtype_fn):
    """[D, N] -> [P, DC_in, N] so each DMA row is contiguous."""
    w = np.asarray(w, np.float32)
    d, n = w.shape
    c = d // P
    return dtype_fn(w.reshape(c, P, n).transpose(1, 0, 2).copy())


def prepare_inputs(inputs):
    x = np.asarray(inputs["x"], dtype=np.float32)

    # LN2 fold: w1_eff = w1 * g2 (per input row), w1gs = -sum_d w1_eff[d,h],
    # b1_eff = b1 + w1.T @ ln2_b
    w1f = np.asarray(inputs["w1"], np.float32)
    g2 = np.asarray(inputs["ln2_g"], np.float32)
    bln2 = np.asarray(inputs["ln2_b"], np.float32)
    w1_eff = w1f * g2[:, None]
    w1gs_neg = -w1_eff.sum(axis=0)
    b1_eff = np.asarray(inputs["b1"], np.float32) + w1f.T @ bln2
    w1t = (_bf(w1_eff).reshape(DC, P, HC, P).transpose(2, 1, 0, 3).copy())
    shared = {
        "wq": _tile_pcn(inputs["wq"], _bf),
        "wk": _tile_pcn(inputs["wk"], _bf),
        "wv": _tile_pcn(inputs["wv"], _bf),
        "wo": _tile_pcn(inputs["wo"], _f8),
        "w1": w1t, "w1gs": w1gs_neg.astype(np.float32),
        "w2": _tile_pcn(inputs["w2"], _bf),
        "ln1_g": np.asarray(inputs["ln1_g"], np.float32),
        "ln1_b": np.asarray(inputs["ln1_b"], np.float32),
        "b1": b1_eff.astype(np.float32),
        "b2": np.asarray(inputs["b2"], np.float32),
    }
    return [dict(shared, x=np.ascontiguousarray(x[i])) for i in range(N_CORES)]


def kernel(**inputs):
    from concourse.bass_utils import run_bass_kernel_spmd

    nc = _get_nc()
    in_maps = prepare_inputs(inputs)
    res = run_bass_kernel_spmd(nc, in_maps, list(range(N_CORES)))
    out = np.stack([res.results[i]["out"] for i in range(N_CORES)], axis=0)
    return out.astype(np.float32)
